# revision 1
# baseline (speedup 1.0000x reference)
"""Trainium2 Bass kernel for AttentionBasedTimestamps.

Computes, from a stacked attention tensor [L=4, B=4, H=16, S=1024, S=1024]:
  avg = mean over (L, H) -> [B, S, S]
  w   = avg[:, 576:, 64:576]                  # [B, T=448, F=512]
  start/end frame (threshold mask first/last index, argmax fallback),
  start/end ms, and a softmax-entropy confidence per (b, t) row.

Sharding: data-parallel over (batch, row-half): core c handles batch c//2
and rows [224*(c%2), 224*(c%2)+224) of the T=448 text rows.

Device layout: the 512 audio columns ride the 128 SBUF partitions during
streaming (s = q*128 + p, 4 quarters) because DMA bandwidth collapses below
128 partitions (~211 GB/s at 112 vs ~360 GB/s at 128 measured). The host
pre-permutes each core's block to [q=4, tb=2, p=128(s), t=112, j=64] so all
eight 3.5MB input DMAs are fully dense. The (l,h)-mean is one dense
innermost-axis DVE reduce per piece into acc[128(s), 4(q), 224(t)]; a PE
transpose (identity matmul) then flips acc to rows-on-partitions for the
row-wise stats.
"""

import sys

import numpy as np

try:  # concourse ships in the runtime image; fall back to the repo path
    import concourse  # noqa: F401
except ImportError:  # pragma: no cover
    sys.path.insert(0, "/opt/trn_rl_repo")

L, B, H, S = 4, 4, 16, 1024
AUDIO_START, AUDIO_END, TEXT_START = 64, 576, 576
FRAME_MS = 40.0
T = S - TEXT_START  # 448 text rows
F = AUDIO_END - AUDIO_START  # 512 audio frames
NS = L * H  # 64 (layer, head) slices averaged
N_CORES = 8
HALVES = 2  # cores per batch
RPC = T // HALVES  # 224 rows per core
NQ = F // 128  # 4 s-quarters
TBLK = 2  # t-blocks per quarter during streaming
TB = RPC // TBLK  # 112 rows per streaming piece
BIG = 1.0e6
INP_BUFS = 4  # input tile buffering depth
DMA_ALT = True  # alternate input DMAs across the SP and ACT HWDGE rings

_cache: dict = {}


def _build_nc(repeat: int = 1, repeat_stats: int = 1):
    import concourse.bacc as bacc
    import concourse.mybir as mybir
    import concourse.tile as tile

    f32 = mybir.dt.float32
    i32 = mybir.dt.int32
    Alu = mybir.AluOpType
    Act = mybir.ActivationFunctionType
    X = mybir.AxisListType.X

    inv_ns = 1.0 / NS
    inv_logf = float(1.0 / np.log(np.float32(F)))

    nc = bacc.Bacc(
        "TRN2", target_bir_lowering=False, debug=False, num_devices=N_CORES
    )
    x = nc.dram_tensor("x", [NQ, TBLK, 128, TB, NS], f32, kind="ExternalInput")
    o_i = nc.dram_tensor("o_i", [RPC, 2], i32, kind="ExternalOutput")
    o_f = nc.dram_tensor("o_f", [RPC, 3], f32, kind="ExternalOutput")
    ident_dram = nc.inline_tensor(np.eye(128, dtype=np.float32), name="ident")

    with tile.TileContext(nc) as tc:
        with (
            tc.tile_pool(name="inp", bufs=INP_BUFS) as inp,
            tc.tile_pool(name="accp", bufs=1) as accp,
            tc.tile_pool(name="wp", bufs=1) as wp,
            tc.tile_pool(name="psum", bufs=2, space="PSUM") as psum,
            tc.tile_pool(name="work", bufs=2) as work,
            tc.tile_pool(name="small", bufs=2) as small,
        ):
            acc = accp.tile([128, NQ, RPC], f32, tag="acc")
            iota = accp.tile([128, F], f32, tag="iota")
            nc.gpsimd.iota(
                iota[:],
                pattern=[[1, F]],
                base=0,
                channel_multiplier=0,
                allow_small_or_imprecise_dtypes=True,
            )
            # iota - 1000: lets masked-min chains use 0 as the "unmasked"
            # value ((A>thr)*iom is negative iff masked), so first/peak are
            # single scalar_tensor_tensor passes; +1000 is re-added at the end.
            iom = accp.tile([128, F], f32, tag="iom")
            nc.gpsimd.iota(
                iom[:],
                pattern=[[1, F]],
                base=-1000,
                channel_multiplier=0,
                allow_small_or_imprecise_dtypes=True,
            )
            ident = accp.tile([128, 128], f32, tag="ident")
            nc.sync.dma_start(ident[:], ident_dram[:])
            w1 = wp.tile([128, F], f32, tag="w1")
            w2 = wp.tile([RPC - 128, F], f32, tag="w2")

            def transpose_quarter(q):
                # PE transpose quarter q: acc[s_p, q, t] -> w[t, s]
                pt1 = psum.tile([128, 128], f32, tag="pt1")
                nc.tensor.transpose(pt1[:], acc[:, q, 0:128], ident[:])
                nc.vector.tensor_copy(w1[:, q * 128 : (q + 1) * 128], pt1[:])
                pt2 = psum.tile([RPC - 128, 128], f32, tag="pt2")
                nc.tensor.transpose(pt2[:], acc[:, q, 128:RPC], ident[:])
                nc.vector.tensor_copy(w2[:, q * 128 : (q + 1) * 128], pt2[:])

            # streaming phase: 8 dense 3.5MB DMAs + innermost-j DVE reduces.
            # `repeat`/`repeat_stats` re-run phases for slope-based timing.
            dma_engines = [nc.sync, nc.scalar] if DMA_ALT else [nc.sync, nc.sync]
            di = 0
            for _r in range(repeat):
                for q in range(NQ):
                    for tb in range(TBLK):
                        last_piece = (
                            _r == repeat - 1 and q == NQ - 1 and tb == TBLK - 1
                        )
                        if not last_piece:
                            t = inp.tile([128, TB, NS], f32, tag="in")
                            dma_engines[di % 2].dma_start(t[:], x[q, tb])
                            di += 1
                            nc.vector.tensor_reduce(
                                acc[:, q, tb * TB : (tb + 1) * TB],
                                t[:],
                                axis=X,
                                op=Alu.add,
                            )
                        else:
                            # split the final piece in two so the last DMA's
                            # dependent reduce (critical path into the tail)
                            # covers half the rows
                            hb = TB // 2
                            for hh in range(2):
                                th = inp.tile([128, hb, NS], f32, tag="inh")
                                dma_engines[di % 2].dma_start(
                                    th[:], x[q, tb, :, hh * hb : (hh + 1) * hb, :]
                                )
                                di += 1
                                r0 = tb * TB + hh * hb
                                nc.vector.tensor_reduce(
                                    acc[:, q, r0 : r0 + hb],
                                    th[:],
                                    axis=X,
                                    op=Alu.add,
                                )
                    if _r == repeat - 1 and repeat_stats == 1:
                        transpose_quarter(q)

            # row-wise stats on [pc, F] tiles (pc = 128 then 96)
            def row_stats(w, pc, off):
                A = w[:]
                amax = small.tile([pc, 1], f32, tag="amax")
                nc.vector.tensor_reduce(amax[:], A, axis=X, op=Alu.max)
                nbias = small.tile([pc, 1], f32, tag="nbias")
                nc.vector.tensor_scalar_mul(nbias[:], amax[:], -inv_ns)
                half = small.tile([pc, 1], f32, tag="half")
                nc.vector.tensor_scalar_mul(half[:], amax[:], 0.5)
                ha = small.tile([pc, 1], i32, tag="ha")
                nc.vector.tensor_single_scalar(ha[:], amax[:], 0.0, Alu.is_gt)
                oi = small.tile([pc, 2], i32, tag="oi")
                of = small.tile([pc, 3], f32, tag="of")

                # confidence via the softmax identity:
                #   sum p*ln p = (sum (A-amax)*e)/(64*Z) - ln Z
                # (exact: A/64 and amax/64 are power-of-two scalings), plus the
                # reference's +1e-9 inside the log as a constant F*1e-9 term.
                # scalar_tensor_tensor's accum_out gives the sum in the same pass.
                e = work.tile([pc, F], f32, tag="e")
                zsum = small.tile([pc, 1], f32, tag="zsum")
                nc.scalar.activation(
                    e[:], A, Act.Exp, bias=nbias[:], scale=inv_ns, accum_out=zsum[:]
                )
                rz = small.tile([pc, 1], f32, tag="rz")
                nc.vector.reciprocal(rz[:], zsum[:])
                gg = work.tile([pc, F], f32, tag="gg")
                u64 = small.tile([pc, 1], f32, tag="u64")
                nc.vector.scalar_tensor_tensor(
                    gg[:], A, amax[:], e[:], Alu.subtract, Alu.mult, accum_out=u64[:]
                )
                lnz = small.tile([pc, 1], f32, tag="lnz")
                nc.scalar.activation(lnz[:], zsum[:], Act.Ln, bias=0.0)
                s1 = small.tile([pc, 1], f32, tag="s1")
                nc.vector.tensor_single_scalar(s1[:], u64[:], rz[:], Alu.mult)
                sv = small.tile([pc, 1], f32, tag="sv")
                nc.vector.scalar_tensor_tensor(
                    sv[:], s1[:], inv_ns, lnz[:], Alu.mult, Alu.subtract
                )
                nc.vector.tensor_scalar(
                    of[:, 2:3],
                    sv[:],
                    inv_logf,
                    1.0 + F * 1e-9 * inv_logf,
                    op0=Alu.mult,
                    op1=Alu.add,
                )

                # mask = A > 0.5*amax: first/last active index, first-argmax.
                # (mask * iom) is negative iff masked, 0 otherwise, so min gives
                # the first masked index - 1000; (mask * iota) max gives the last.
                t1 = work.tile([pc, F], f32, tag="t1")
                nc.vector.scalar_tensor_tensor(
                    t1[:], A, half[:], iom[:pc, :], Alu.is_gt, Alu.mult
                )
                fi = small.tile([pc, 1], f32, tag="fi")
                nc.vector.tensor_reduce(fi[:], t1[:], axis=X, op=Alu.min)
                t2 = work.tile([pc, F], f32, tag="t2")
                nc.vector.scalar_tensor_tensor(
                    t2[:], A, half[:], iota[:pc, :], Alu.is_gt, Alu.mult
                )
                la = small.tile([pc, 1], f32, tag="la")
                nc.vector.tensor_reduce(la[:], t2[:], axis=X, op=Alu.max)
                t3 = work.tile([pc, F], f32, tag="t3")
                nc.vector.scalar_tensor_tensor(
                    t3[:], A, amax[:], iom[:pc, :], Alu.is_equal, Alu.mult
                )
                pk = small.tile([pc, 1], f32, tag="pk")
                nc.vector.tensor_reduce(pk[:], t3[:], axis=X, op=Alu.min)
                pkt = small.tile([pc, 1], f32, tag="pkt")
                nc.vector.tensor_scalar_add(pkt[:], pk[:], 1000.0)

                # start/end on the -1000 / true bases respectively
                stf = small.tile([pc, 1], f32, tag="stf")
                nc.vector.select(stf[:], ha[:], fi[:], pk[:])
                enf = small.tile([pc, 1], f32, tag="enf")
                nc.vector.select(enf[:], ha[:], la[:], pkt[:])

                # pack outputs: [sf, ef] i32 and [sms, ems, conf] f32
                nc.vector.tensor_scalar_add(oi[:, 0:1], stf[:], 1000.0)
                nc.vector.tensor_copy(oi[:, 1:2], enf[:])
                nc.vector.tensor_scalar(
                    of[:, 0:1], stf[:], FRAME_MS, 1000.0 * FRAME_MS,
                    op0=Alu.mult, op1=Alu.add,
                )
                nc.vector.tensor_scalar_mul(of[:, 1:2], enf[:], FRAME_MS)

                nc.sync.dma_start(o_i[off : off + pc, :], oi[:])
                nc.sync.dma_start(o_f[off : off + pc, :], of[:])

            for _rs in range(repeat_stats):
                if repeat_stats > 1:
                    for q in range(NQ):
                        transpose_quarter(q)
                row_stats(w1, 128, 0)
                row_stats(w2, RPC - 128, 128)

    nc.compile()
    return nc


def _get_nc():
    if "nc" not in _cache:
        _cache["nc"] = _build_nc()
    return _cache["nc"]


def _prep_in_maps(attn: np.ndarray) -> list[dict]:
    sub = attn[:, :, :, TEXT_START:, AUDIO_START:AUDIO_END]  # [L,B,H,T,F]
    in_maps = []
    for c in range(N_CORES):
        b, hf = divmod(c, HALVES)
        blk = sub[:, b, :, hf * RPC : (hf + 1) * RPC, :]  # [L,H,RPC,F]
        # -> [q, s_p, tb, t, j] -> [q, tb, s_p, t, j], fully dense per DMA
        xc = (
            blk.transpose(3, 2, 0, 1)
            .reshape(NQ, 128, TBLK, TB, NS)
            .transpose(0, 2, 1, 3, 4)
        )
        in_maps.append({"x": np.ascontiguousarray(xc)})
    return in_maps


def _run(in_maps, trace=False, **kw):
    from concourse.bass_utils import run_bass_kernel_spmd

    return run_bass_kernel_spmd(
        _get_nc(), in_maps, list(range(N_CORES)), trace=trace, **kw
    )


def _assemble(results):
    sf = np.empty((B, T), np.int32)
    ef = np.empty((B, T), np.int32)
    sms = np.empty((B, T), np.float32)
    ems = np.empty((B, T), np.float32)
    conf = np.empty((B, T), np.float32)
    for c in range(N_CORES):
        b, hf = divmod(c, HALVES)
        rows = slice(hf * RPC, (hf + 1) * RPC)
        r = results[c]
        sf[b, rows] = r["o_i"][:, 0]
        ef[b, rows] = r["o_i"][:, 1]
        sms[b, rows] = r["o_f"][:, 0]
        ems[b, rows] = r["o_f"][:, 1]
        conf[b, rows] = r["o_f"][:, 2]
    return sf, ef, sms, ems, conf


def _reference_numpy(attn, a0, a1, t0):
    """Shape-general CPU fallback mirroring the reference semantics."""
    avg = attn.astype(np.float32).mean(axis=(0, 2))
    w = avg[:, t0:, a0:a1]
    nf = w.shape[-1]
    wmax = w.max(axis=-1, keepdims=True)
    peak = w.argmax(axis=-1)
    mask = w > 0.5 * wmax
    has = mask.any(axis=-1)
    first = mask.argmax(axis=-1)
    last = nf - 1 - mask[..., ::-1].argmax(axis=-1)
    startf = np.where(has, first, peak).astype(np.int32)
    endf = np.where(has, last, peak).astype(np.int32)
    m = w.max(axis=-1, keepdims=True)
    ez = np.exp(w - m)
    probs = ez / ez.sum(axis=-1, keepdims=True)
    ent = -(probs * np.log(probs + 1e-9)).sum(axis=-1)
    confv = (1.0 - ent / np.log(np.float32(nf))).astype(np.float32)
    return (
        startf,
        endf,
        (startf * np.float32(FRAME_MS)).astype(np.float32),
        (endf * np.float32(FRAME_MS)).astype(np.float32),
        confv,
    )


def kernel(
    attentions,
    audio_start_idx=AUDIO_START,
    audio_end_idx=AUDIO_END,
    text_start_idx=TEXT_START,
    **_unused,
):
    attn = np.asarray(attentions, dtype=np.float32)
    a0 = int(np.asarray(audio_start_idx))
    a1 = int(np.asarray(audio_end_idx))
    t0 = int(np.asarray(text_start_idx))
    if attn.shape != (L, B, H, S, S) or (a0, a1, t0) != (
        AUDIO_START,
        AUDIO_END,
        TEXT_START,
    ):
        return _reference_numpy(attn, a0, a1, t0)
    in_maps = _prep_in_maps(attn)
    try:
        res = _run(in_maps)
    except Exception:
        try:  # one retry: a wedged exec unit usually recovers on redispatch
            res = _run(in_maps)
        except Exception as ex:  # noqa: BLE001
            sys.stderr.write(f"kernel: device path failed ({ex!r}); CPU fallback\n")
            return _reference_numpy(attn, a0, a1, t0)
    return _assemble(res.results)



# revision 2
# speedup vs baseline: 1.1673x; 1.1673x over previous
"""Trainium2 Bass kernel v2 for AttentionBasedTimestamps.

Differences from v1 (kernel.py):
  * Input shipped as CENTERED bf16 (x - 0.5): halves HBM traffic; softmax/
    argmax are shift-invariant, only the 0.5*max threshold needs a -16
    constant (0.5 * NS * shift) and has_active becomes amax' > -32.
  * lh-major free layout [128 f | 64 lh | TB t] so the (l,h)-sum runs as a
    binary tree of CONTIGUOUS bf16 tensor_tensor adds (2x DVE packing) or a
    single grouped reduce, selected by REDUCE_MODE.
  * t-block OUTER loop: each t-block streams its 4 f-quarters, transposes,
    and runs row-stats while the next t-block streams -> stats tail mostly
    hidden.

Sharding: core c handles batch c//2, rows [224*(c%2), 224*(c%2)+224).
"""

import sys

import numpy as np

try:
    import concourse  # noqa: F401
except ImportError:  # pragma: no cover
    sys.path.insert(0, "/opt/trn_rl_repo")

import ml_dtypes

L, B, H, S = 4, 4, 16, 1024
AUDIO_START, AUDIO_END, TEXT_START = 64, 576, 576
FRAME_MS = 40.0
T = S - TEXT_START  # 448
F = AUDIO_END - AUDIO_START  # 512
NS = L * H  # 64
N_CORES = 8
HALVES = 2
RPC = T // HALVES  # 224
NQ = F // 128  # 4
TBLK = 2
TB = RPC // TBLK  # 112
SHIFT = 0.5
THR_ADJ = -0.5 * NS * SHIFT  # -16: A' > 0.5*Amax' + THR_ADJ
HA_THR = -float(NS * SHIFT)  # amax' > -32 <=> has_active
REDUCE_MODE = "combo"  # "tree" | "grouped" | "combo"
DMA_RINGS = 1  # inputs: sync ring only; outputs ride the scalar ring
# Quarters reduced on the Tensor engine via PSUM-accumulated transposes
# (lands pre-transposed in the stats tile; frees DVE time). Others use the
# DVE combo path.
PE_QUARTERS = frozenset()  # bf16 transpose needs bf16 PSUM; disabled

_cache: dict = {}


def _build_nc(repeat: int = 1):
    import concourse.bacc as bacc
    import concourse.mybir as mybir
    import concourse.tile as tile

    f32 = mybir.dt.float32
    bf16 = mybir.dt.bfloat16
    i32 = mybir.dt.int32
    Alu = mybir.AluOpType
    Act = mybir.ActivationFunctionType
    X = mybir.AxisListType.X

    inv_ns = 1.0 / NS
    inv_logf = float(1.0 / np.log(np.float32(F)))

    nc = bacc.Bacc(
        "TRN2", target_bir_lowering=False, debug=False, num_devices=N_CORES
    )
    x = nc.dram_tensor("x", [TBLK, NQ, 128, NS, TB], bf16, kind="ExternalInput")
    o_i = nc.dram_tensor("o_i", [RPC, 2], i32, kind="ExternalOutput")
    o_f = nc.dram_tensor("o_f", [RPC, 3], f32, kind="ExternalOutput")
    ident_dram = nc.inline_tensor(np.eye(128, dtype=np.float32), name="ident")

    with tile.TileContext(nc) as tc:
        with (
            tc.tile_pool(name="inp", bufs=4) as inp,
            tc.tile_pool(name="trp", bufs=2) as trp,
            tc.tile_pool(name="accp", bufs=2) as accp,
            tc.tile_pool(name="psum", bufs=2, space="PSUM") as psum,
            tc.tile_pool(name="work", bufs=2) as work,
            tc.tile_pool(name="small", bufs=2) as small,
            tc.tile_pool(name="constp", bufs=1) as constp,
            nc.allow_low_precision(reason="bf16 tree partial sums; final acc f32"),
        ):
            iota = constp.tile([128, F], f32, tag="iota")
            nc.gpsimd.iota(
                iota[:],
                pattern=[[1, F]],
                base=0,
                channel_multiplier=0,
                allow_small_or_imprecise_dtypes=True,
            )
            iom = constp.tile([128, F], f32, tag="iom")
            nc.gpsimd.iota(
                iom[:],
                pattern=[[1, F]],
                base=-1000,
                channel_multiplier=0,
                allow_small_or_imprecise_dtypes=True,
            )
            ident = constp.tile([128, 128], f32, tag="ident")
            nc.sync.dma_start(ident[:], ident_dram[:])
            identb = constp.tile([128, 128], bf16, tag="identb")
            nc.vector.tensor_copy(identb[:], ident[:])

            dma_engines = [nc.sync, nc.scalar, nc.gpsimd][:DMA_RINGS]
            di = 0

            def stream_piece(tb, q, acc_q, w):
                nonlocal di
                t = inp.tile([128, 2, TB * 32], bf16, tag="in")
                dma_engines[di % DMA_RINGS].dma_start(t[:], x[tb, q])
                di += 1
                if (tb, q) in PE_QUARTERS:
                    # lh-major piece [128 f, NS lh, TB t]: 64 PSUM-accumulated
                    # PE transposes land the lh-sum pre-transposed in w.
                    tf = t[:].rearrange("p a b -> p (a b)")
                    for lh in range(NS):
                        nc.tensor.matmul(
                            w[:, q * 128 : (q + 1) * 128],
                            tf[:, lh * TB : (lh + 1) * TB],
                            identb[:],
                            is_transpose=True,
                            start=(lh == 0),
                            stop=(lh == NS - 1),
                        )
                    return
                # combo: piece [128 f, 2 lh-half, TB t, 32 lh-inner]:
                # L1 bf16+bf16 -> f32 upcast-add, then grouped-32 X-reduce.
                # acc_q is a per-quarter tile: consecutive reduces hit
                # different tiles, avoiding same-tile WAW stalls.
                s = trp.tile([128, TB, 32], f32, tag="s")
                sf = s[:].rearrange("p a b -> p (a b)")
                nc.vector.tensor_tensor(sf, t[:, 0, :], t[:, 1, :], Alu.add)
                nc.vector.tensor_reduce(acc_q[:], s[:], axis=X, op=Alu.add)

            def transpose_block(tb, accs, w):
                # PE transposes write disjoint slices of one PSUM tile;
                # stats then read it directly (no DVE copies).
                for q in range(NQ):
                    if (tb, q) in PE_QUARTERS:
                        continue
                    nc.tensor.transpose(
                        w[:, q * 128 : (q + 1) * 128], accs[q][:], ident[:]
                    )

            def row_stats(w, pc, off):
                # Ordered so the scalar-engine Exp runs concurrently with the
                # DVE mask chains (engines execute their streams in program
                # order; the softmax consumers come after the masks so DVE
                # never stalls on the scalar engine).
                A = w[:]
                amax = small.tile([pc, 1], f32, tag="amax")
                nc.vector.tensor_reduce(amax[:], A, axis=X, op=Alu.max)
                nbias = small.tile([pc, 1], f32, tag="nbias")
                nc.vector.tensor_scalar_mul(nbias[:], amax[:], -inv_ns)
                half = small.tile([pc, 1], f32, tag="half")
                nc.vector.tensor_scalar(
                    half[:], amax[:], 0.5, THR_ADJ, op0=Alu.mult, op1=Alu.add
                )
                ha = small.tile([pc, 1], i32, tag="ha")
                nc.vector.tensor_single_scalar(ha[:], amax[:], HA_THR, Alu.is_gt)
                oi = small.tile([pc, 2], i32, tag="oi")
                of = small.tile([pc, 3], f32, tag="of")

                # kick off softmax exp on the scalar engine early
                e = work.tile([pc, F], f32, tag="e")
                zsum = small.tile([pc, 1], f32, tag="zsum")
                nc.scalar.activation(
                    e[:], A, Act.Exp, bias=nbias[:], scale=inv_ns, accum_out=zsum[:]
                )

                # DVE mask chains overlap the Exp
                t1 = work.tile([pc, F], f32, tag="t1")
                nc.vector.scalar_tensor_tensor(
                    t1[:], A, half[:], iom[:pc, :], Alu.is_gt, Alu.mult
                )
                fi = small.tile([pc, 1], f32, tag="fi")
                nc.vector.tensor_reduce(fi[:], t1[:], axis=X, op=Alu.min)
                t2 = work.tile([pc, F], f32, tag="t2")
                nc.vector.scalar_tensor_tensor(
                    t2[:], A, half[:], iota[:pc, :], Alu.is_gt, Alu.mult
                )
                la = small.tile([pc, 1], f32, tag="la")
                nc.vector.tensor_reduce(la[:], t2[:], axis=X, op=Alu.max)
                t3 = work.tile([pc, F], f32, tag="t3")
                nc.vector.scalar_tensor_tensor(
                    t3[:], A, amax[:], iom[:pc, :], Alu.is_equal, Alu.mult
                )
                pk = small.tile([pc, 1], f32, tag="pk")
                nc.vector.tensor_reduce(pk[:], t3[:], axis=X, op=Alu.min)
                pkt = small.tile([pc, 1], f32, tag="pkt")
                nc.vector.tensor_scalar_add(pkt[:], pk[:], 1000.0)

                stf = small.tile([pc, 1], f32, tag="stf")
                nc.vector.select(stf[:], ha[:], fi[:], pk[:])
                enf = small.tile([pc, 1], f32, tag="enf")
                nc.vector.select(enf[:], ha[:], la[:], pkt[:])

                nc.vector.tensor_scalar_add(oi[:, 0:1], stf[:], 1000.0)
                nc.vector.tensor_copy(oi[:, 1:2], enf[:])
                nc.vector.tensor_scalar(
                    of[:, 0:1], stf[:], FRAME_MS, 1000.0 * FRAME_MS,
                    op0=Alu.mult, op1=Alu.add,
                )
                nc.vector.tensor_scalar_mul(of[:, 1:2], enf[:], FRAME_MS)
                nc.scalar.dma_start(o_i[off : off + pc, :], oi[:])

                # softmax entropy consumers (e/zsum long since ready)
                gg = work.tile([pc, F], f32, tag="gg")
                u64 = small.tile([pc, 1], f32, tag="u64")
                nc.vector.scalar_tensor_tensor(
                    gg[:], A, amax[:], e[:], Alu.subtract, Alu.mult, accum_out=u64[:]
                )
                lnz = small.tile([pc, 1], f32, tag="lnz")
                nc.scalar.activation(lnz[:], zsum[:], Act.Ln, bias=0.0)
                rz = small.tile([pc, 1], f32, tag="rz")
                nc.vector.reciprocal(rz[:], zsum[:])
                s1 = small.tile([pc, 1], f32, tag="s1")
                nc.vector.tensor_single_scalar(s1[:], u64[:], rz[:], Alu.mult)
                sv = small.tile([pc, 1], f32, tag="sv")
                nc.vector.scalar_tensor_tensor(
                    sv[:], s1[:], inv_ns, lnz[:], Alu.mult, Alu.subtract
                )
                nc.vector.tensor_scalar(
                    of[:, 2:3],
                    sv[:],
                    inv_logf,
                    1.0 + F * 1e-9 * inv_logf,
                    op0=Alu.mult,
                    op1=Alu.add,
                )
                nc.scalar.dma_start(o_f[off : off + pc, :], of[:])

            for _r in range(repeat):
                for tb in range(TBLK):
                    accs = [
                        accp.tile([128, TB], f32, tag=f"acc{q}", name=f"acc{q}")
                        for q in range(NQ)
                    ]
                    w = psum.tile([TB, F], f32, tag="w")
                    for q in range(NQ):
                        stream_piece(tb, q, accs[q], w)
                    transpose_block(tb, accs, w)
                    row_stats(w, TB, tb * TB)

    nc.compile()
    return nc


def _get_nc():
    if "nc" not in _cache:
        _cache["nc"] = _build_nc()
    return _cache["nc"]


def _prep_in_maps(attn: np.ndarray) -> list[dict]:
    sub = attn[:, :, :, TEXT_START:, AUDIO_START:AUDIO_END]  # [L,B,H,T,F]
    in_maps = []
    for c in range(N_CORES):
        b, hf = divmod(c, HALVES)
        blk = sub[:, b, :, hf * RPC : (hf + 1) * RPC, :]  # [L,H,RPC,F]
        base = (blk.reshape(NS, TBLK, TB, NQ, 128).astype(np.float32) - SHIFT).astype(
            ml_dtypes.bfloat16
        )  # [lh, tb, t, q, f]
        out = np.empty((TBLK, NQ, 128, NS * TB), ml_dtypes.bfloat16)
        for tb in range(TBLK):
            for q in range(NQ):
                piece = base[:, tb, :, q, :]  # [NS, TB, 128]
                if (tb, q) in PE_QUARTERS:
                    # lh-major [f_p, lh, t] for PE transpose-accumulate
                    p = piece.transpose(2, 0, 1)
                else:
                    # combo [f_p, ho, t, li]
                    p = piece.reshape(2, 32, TB, 128).transpose(3, 0, 2, 1)
                out[tb, q] = np.ascontiguousarray(p).reshape(128, NS * TB)
        in_maps.append({"x": out})
    return in_maps


def _run(in_maps, trace=False, **kw):
    from concourse.bass_utils import run_bass_kernel_spmd

    return run_bass_kernel_spmd(
        _get_nc(), in_maps, list(range(N_CORES)), trace=trace, **kw
    )


def _assemble(results):
    sf = np.empty((B, T), np.int32)
    ef = np.empty((B, T), np.int32)
    sms = np.empty((B, T), np.float32)
    ems = np.empty((B, T), np.float32)
    conf = np.empty((B, T), np.float32)
    for c in range(N_CORES):
        b, hf = divmod(c, HALVES)
        rows = slice(hf * RPC, (hf + 1) * RPC)
        r = results[c]
        sf[b, rows] = r["o_i"][:, 0]
        ef[b, rows] = r["o_i"][:, 1]
        sms[b, rows] = r["o_f"][:, 0]
        ems[b, rows] = r["o_f"][:, 1]
        conf[b, rows] = r["o_f"][:, 2]
    return sf, ef, sms, ems, conf


def _reference_numpy(attn, a0, a1, t0):
    avg = attn.astype(np.float32).mean(axis=(0, 2))
    w = avg[:, t0:, a0:a1]
    nf = w.shape[-1]
    wmax = w.max(axis=-1, keepdims=True)
    peak = w.argmax(axis=-1)
    mask = w > 0.5 * wmax
    has = mask.any(axis=-1)
    first = mask.argmax(axis=-1)
    last = nf - 1 - mask[..., ::-1].argmax(axis=-1)
    startf = np.where(has, first, peak).astype(np.int32)
    endf = np.where(has, last, peak).astype(np.int32)
    m = w.max(axis=-1, keepdims=True)
    ez = np.exp(w - m)
    probs = ez / ez.sum(axis=-1, keepdims=True)
    ent = -(probs * np.log(probs + 1e-9)).sum(axis=-1)
    confv = (1.0 - ent / np.log(np.float32(nf))).astype(np.float32)
    return (
        startf,
        endf,
        (startf * np.float32(FRAME_MS)).astype(np.float32),
        (endf * np.float32(FRAME_MS)).astype(np.float32),
        confv,
    )


def kernel(
    attentions,
    audio_start_idx=AUDIO_START,
    audio_end_idx=AUDIO_END,
    text_start_idx=TEXT_START,
    **_unused,
):
    attn = np.asarray(attentions, dtype=np.float32)
    a0 = int(np.asarray(audio_start_idx))
    a1 = int(np.asarray(audio_end_idx))
    t0 = int(np.asarray(text_start_idx))
    if attn.shape != (L, B, H, S, S) or (a0, a1, t0) != (
        AUDIO_START,
        AUDIO_END,
        TEXT_START,
    ):
        return _reference_numpy(attn, a0, a1, t0)
    in_maps = _prep_in_maps(attn)
    try:
        res = _run(in_maps)
    except Exception:
        try:
            res = _run(in_maps)
        except Exception as ex:  # noqa: BLE001
            sys.stderr.write(f"kernel: device path failed ({ex!r}); CPU fallback\n")
            return _reference_numpy(attn, a0, a1, t0)
    return _assemble(res.results)


# revision 3
# speedup vs baseline: 1.5285x; 1.3095x over previous
"""Trainium2 Bass kernel v2 for AttentionBasedTimestamps.

Differences from v1 (kernel.py):
  * Input shipped as CENTERED bf16 (x - 0.5): halves HBM traffic; softmax/
    argmax are shift-invariant, only the 0.5*max threshold needs a -16
    constant (0.5 * NS * shift) and has_active becomes amax' > -32.
  * lh-major free layout [128 f | 64 lh | TB t] so the (l,h)-sum runs as a
    binary tree of CONTIGUOUS bf16 tensor_tensor adds (2x DVE packing) or a
    single grouped reduce, selected by REDUCE_MODE.
  * t-block OUTER loop: each t-block streams its 4 f-quarters, transposes,
    and runs row-stats while the next t-block streams -> stats tail mostly
    hidden.

Sharding: core c handles batch c//2, rows [224*(c%2), 224*(c%2)+224).
"""

import sys

import numpy as np

try:
    import concourse  # noqa: F401
except ImportError:  # pragma: no cover
    sys.path.insert(0, "/opt/trn_rl_repo")

import ml_dtypes

L, B, H, S = 4, 4, 16, 1024
AUDIO_START, AUDIO_END, TEXT_START = 64, 576, 576
FRAME_MS = 40.0
T = S - TEXT_START  # 448
F = AUDIO_END - AUDIO_START  # 512
NS = L * H  # 64
N_CORES = 8
HALVES = 2
RPC = T // HALVES  # 224
NQ = F // 128  # 4
TBLK = 2
TB = RPC // TBLK  # 112
SHIFT = 0.5
THR_ADJ = -0.5 * NS * SHIFT  # -16: A' > 0.5*Amax' + THR_ADJ
HA_THR = -float(NS * SHIFT)  # amax' > -32 <=> has_active
REDUCE_MODE = "combo"  # "tree" | "grouped" | "combo"
DMA_RINGS = 1  # inputs: sync ring only; outputs ride the scalar ring
# Quarters reduced on the Tensor engine via PSUM-accumulated transposes
# (lands pre-transposed in the stats tile; frees DVE time). Others use the
# DVE combo path.
PE_QUARTERS = frozenset()  # bf16 transpose needs bf16 PSUM; disabled

_cache: dict = {}


def _build_nc(repeat: int = 1):
    import concourse.bacc as bacc
    import concourse.mybir as mybir
    import concourse.tile as tile

    f32 = mybir.dt.float32
    bf16 = mybir.dt.bfloat16
    i32 = mybir.dt.int32
    Alu = mybir.AluOpType
    Act = mybir.ActivationFunctionType
    X = mybir.AxisListType.X

    inv_ns = 1.0 / NS
    inv_logf = float(1.0 / np.log(np.float32(F)))

    nc = bacc.Bacc(
        "TRN2", target_bir_lowering=False, debug=False, num_devices=N_CORES
    )
    x = nc.dram_tensor("x", [TBLK, NQ, 128, NS, TB], bf16, kind="ExternalInput")
    o_i = nc.dram_tensor("o_i", [RPC, 2], i32, kind="ExternalOutput")
    o_f = nc.dram_tensor("o_f", [RPC, 3], f32, kind="ExternalOutput")
    ident_dram = nc.inline_tensor(np.eye(128, dtype=np.float32), name="ident")

    with tile.TileContext(nc) as tc:
        with (
            tc.tile_pool(name="inp", bufs=6) as inp,
            tc.tile_pool(name="trp", bufs=2) as trp,
            tc.tile_pool(name="accp", bufs=2) as accp,
            tc.tile_pool(name="psum", bufs=2, space="PSUM") as psum,
            tc.tile_pool(name="work", bufs=2) as work,
            tc.tile_pool(name="small", bufs=2) as small,
            tc.tile_pool(name="constp", bufs=1) as constp,
            nc.allow_low_precision(reason="bf16 tree partial sums; final acc f32"),
        ):
            iota = constp.tile([128, F], f32, tag="iota")
            nc.gpsimd.iota(
                iota[:],
                pattern=[[1, F]],
                base=0,
                channel_multiplier=0,
                allow_small_or_imprecise_dtypes=True,
            )
            iom = constp.tile([128, F], f32, tag="iom")
            nc.gpsimd.iota(
                iom[:],
                pattern=[[1, F]],
                base=-1000,
                channel_multiplier=0,
                allow_small_or_imprecise_dtypes=True,
            )
            ident = constp.tile([128, 128], f32, tag="ident")
            nc.sync.dma_start(ident[:], ident_dram[:])
            identb = constp.tile([128, 128], bf16, tag="identb")
            nc.vector.tensor_copy(identb[:], ident[:])

            dma_engines = [nc.sync, nc.scalar, nc.gpsimd][:DMA_RINGS]
            di = 0

            def stream_piece(tb, q, acc_q, w):
                nonlocal di
                t = inp.tile([128, 2, TB * 32], bf16, tag="in")
                dma_engines[di % DMA_RINGS].dma_start(t[:], x[tb, q])
                di += 1
                if (tb, q) in PE_QUARTERS:
                    # lh-major piece [128 f, NS lh, TB t]: 64 PSUM-accumulated
                    # PE transposes land the lh-sum pre-transposed in w.
                    tf = t[:].rearrange("p a b -> p (a b)")
                    for lh in range(NS):
                        nc.tensor.matmul(
                            w[:, q * 128 : (q + 1) * 128],
                            tf[:, lh * TB : (lh + 1) * TB],
                            identb[:],
                            is_transpose=True,
                            start=(lh == 0),
                            stop=(lh == NS - 1),
                        )
                    return
                # combo: piece [128 f, 2 lh-half, TB t, 32 lh-inner]:
                # L1 bf16+bf16 -> f32 upcast-add, then grouped-32 X-reduce.
                # acc_q is a per-quarter tile: consecutive reduces hit
                # different tiles, avoiding same-tile WAW stalls.
                s = trp.tile([128, TB, 32], f32, tag="s")
                sf = s[:].rearrange("p a b -> p (a b)")
                nc.vector.tensor_tensor(sf, t[:, 0, :], t[:, 1, :], Alu.add)
                nc.vector.tensor_reduce(acc_q[:], s[:], axis=X, op=Alu.add)

            def transpose_block(tb, accs, w):
                # PE transposes write disjoint slices of one PSUM tile;
                # stats then read it directly (no DVE copies).
                for q in range(NQ):
                    if (tb, q) in PE_QUARTERS:
                        continue
                    nc.tensor.transpose(
                        w[:, q * 128 : (q + 1) * 128], accs[q][:], ident[:]
                    )

            def row_stats(w, pc, off):
                # Ordered so the scalar-engine Exp runs concurrently with the
                # DVE mask chains (engines execute their streams in program
                # order; the softmax consumers come after the masks so DVE
                # never stalls on the scalar engine).
                A = w[:]
                amax = small.tile([pc, 1], f32, tag="amax")
                nc.vector.tensor_reduce(amax[:], A, axis=X, op=Alu.max)
                nbias = small.tile([pc, 1], f32, tag="nbias")
                nc.vector.tensor_scalar_mul(nbias[:], amax[:], -inv_ns)
                half = small.tile([pc, 1], f32, tag="half")
                nc.vector.tensor_scalar(
                    half[:], amax[:], 0.5, THR_ADJ, op0=Alu.mult, op1=Alu.add
                )
                ha = small.tile([pc, 1], i32, tag="ha")
                nc.vector.tensor_single_scalar(ha[:], amax[:], HA_THR, Alu.is_gt)
                oi = small.tile([pc, 2], i32, tag="oi")
                of = small.tile([pc, 3], f32, tag="of")

                # kick off softmax exp on the scalar engine early
                e = work.tile([pc, F], f32, tag="e")
                zsum = small.tile([pc, 1], f32, tag="zsum")
                nc.scalar.activation(
                    e[:], A, Act.Exp, bias=nbias[:], scale=inv_ns, accum_out=zsum[:]
                )

                # DVE mask chains overlap the Exp
                t1 = work.tile([pc, F], f32, tag="t1")
                nc.vector.scalar_tensor_tensor(
                    t1[:], A, half[:], iom[:pc, :], Alu.is_gt, Alu.mult
                )
                fi = small.tile([pc, 1], f32, tag="fi")
                nc.vector.tensor_reduce(fi[:], t1[:], axis=X, op=Alu.min)
                t2 = work.tile([pc, F], f32, tag="t2")
                nc.vector.scalar_tensor_tensor(
                    t2[:], A, half[:], iota[:pc, :], Alu.is_gt, Alu.mult
                )
                la = small.tile([pc, 1], f32, tag="la")
                nc.vector.tensor_reduce(la[:], t2[:], axis=X, op=Alu.max)
                t3 = work.tile([pc, F], f32, tag="t3")
                nc.vector.scalar_tensor_tensor(
                    t3[:], A, amax[:], iom[:pc, :], Alu.is_equal, Alu.mult
                )
                pk = small.tile([pc, 1], f32, tag="pk")
                nc.vector.tensor_reduce(pk[:], t3[:], axis=X, op=Alu.min)
                pkt = small.tile([pc, 1], f32, tag="pkt")
                nc.vector.tensor_scalar_add(pkt[:], pk[:], 1000.0)

                stf = small.tile([pc, 1], f32, tag="stf")
                nc.vector.select(stf[:], ha[:], fi[:], pk[:])
                enf = small.tile([pc, 1], f32, tag="enf")
                nc.vector.select(enf[:], ha[:], la[:], pkt[:])

                nc.vector.tensor_scalar_add(oi[:, 0:1], stf[:], 1000.0)
                nc.vector.tensor_copy(oi[:, 1:2], enf[:])
                nc.vector.tensor_scalar(
                    of[:, 0:1], stf[:], FRAME_MS, 1000.0 * FRAME_MS,
                    op0=Alu.mult, op1=Alu.add,
                )
                nc.vector.tensor_scalar_mul(of[:, 1:2], enf[:], FRAME_MS)
                nc.scalar.dma_start(o_i[off : off + pc, :], oi[:])

                # softmax entropy consumers (e/zsum long since ready)
                gg = work.tile([pc, F], f32, tag="gg")
                u64 = small.tile([pc, 1], f32, tag="u64")
                nc.vector.scalar_tensor_tensor(
                    gg[:], A, amax[:], e[:], Alu.subtract, Alu.mult, accum_out=u64[:]
                )
                lnz = small.tile([pc, 1], f32, tag="lnz")
                nc.scalar.activation(lnz[:], zsum[:], Act.Ln, bias=0.0)
                rz = small.tile([pc, 1], f32, tag="rz")
                nc.vector.reciprocal(rz[:], zsum[:])
                s1 = small.tile([pc, 1], f32, tag="s1")
                nc.vector.tensor_single_scalar(s1[:], u64[:], rz[:], Alu.mult)
                sv = small.tile([pc, 1], f32, tag="sv")
                nc.vector.scalar_tensor_tensor(
                    sv[:], s1[:], inv_ns, lnz[:], Alu.mult, Alu.subtract
                )
                nc.vector.tensor_scalar(
                    of[:, 2:3],
                    sv[:],
                    inv_logf,
                    1.0 + F * 1e-9 * inv_logf,
                    op0=Alu.mult,
                    op1=Alu.add,
                )
                nc.scalar.dma_start(o_f[off : off + pc, :], of[:])

            for _r in range(repeat):
                for tb in range(TBLK):
                    accs = [
                        accp.tile([128, TB], f32, tag=f"acc{q}", name=f"acc{q}")
                        for q in range(NQ)
                    ]
                    w = psum.tile([TB, F], f32, tag="w")
                    for q in range(NQ):
                        stream_piece(tb, q, accs[q], w)
                    transpose_block(tb, accs, w)
                    row_stats(w, TB, tb * TB)

    nc.compile()
    return nc


def _get_nc():
    if "nc" not in _cache:
        _cache["nc"] = _build_nc()
    return _cache["nc"]


def _prep_in_maps(attn: np.ndarray) -> list[dict]:
    sub = attn[:, :, :, TEXT_START:, AUDIO_START:AUDIO_END]  # [L,B,H,T,F]
    in_maps = []
    for c in range(N_CORES):
        b, hf = divmod(c, HALVES)
        blk = sub[:, b, :, hf * RPC : (hf + 1) * RPC, :]  # [L,H,RPC,F]
        base = (blk.reshape(NS, TBLK, TB, NQ, 128).astype(np.float32) - SHIFT).astype(
            ml_dtypes.bfloat16
        )  # [lh, tb, t, q, f]
        out = np.empty((TBLK, NQ, 128, NS * TB), ml_dtypes.bfloat16)
        for tb in range(TBLK):
            for q in range(NQ):
                piece = base[:, tb, :, q, :]  # [NS, TB, 128]
                if (tb, q) in PE_QUARTERS:
                    # lh-major [f_p, lh, t] for PE transpose-accumulate
                    p = piece.transpose(2, 0, 1)
                else:
                    # combo [f_p, ho, t, li]
                    p = piece.reshape(2, 32, TB, 128).transpose(3, 0, 2, 1)
                out[tb, q] = np.ascontiguousarray(p).reshape(128, NS * TB)
        in_maps.append({"x": out})
    return in_maps


def _run(in_maps, trace=False, **kw):
    from concourse.bass_utils import run_bass_kernel_spmd

    return run_bass_kernel_spmd(
        _get_nc(), in_maps, list(range(N_CORES)), trace=trace, **kw
    )


def _assemble(results):
    sf = np.empty((B, T), np.int32)
    ef = np.empty((B, T), np.int32)
    sms = np.empty((B, T), np.float32)
    ems = np.empty((B, T), np.float32)
    conf = np.empty((B, T), np.float32)
    for c in range(N_CORES):
        b, hf = divmod(c, HALVES)
        rows = slice(hf * RPC, (hf + 1) * RPC)
        r = results[c]
        sf[b, rows] = r["o_i"][:, 0]
        ef[b, rows] = r["o_i"][:, 1]
        sms[b, rows] = r["o_f"][:, 0]
        ems[b, rows] = r["o_f"][:, 1]
        conf[b, rows] = r["o_f"][:, 2]
    return sf, ef, sms, ems, conf


def _reference_numpy(attn, a0, a1, t0):
    avg = attn.astype(np.float32).mean(axis=(0, 2))
    w = avg[:, t0:, a0:a1]
    nf = w.shape[-1]
    wmax = w.max(axis=-1, keepdims=True)
    peak = w.argmax(axis=-1)
    mask = w > 0.5 * wmax
    has = mask.any(axis=-1)
    first = mask.argmax(axis=-1)
    last = nf - 1 - mask[..., ::-1].argmax(axis=-1)
    startf = np.where(has, first, peak).astype(np.int32)
    endf = np.where(has, last, peak).astype(np.int32)
    m = w.max(axis=-1, keepdims=True)
    ez = np.exp(w - m)
    probs = ez / ez.sum(axis=-1, keepdims=True)
    ent = -(probs * np.log(probs + 1e-9)).sum(axis=-1)
    confv = (1.0 - ent / np.log(np.float32(nf))).astype(np.float32)
    return (
        startf,
        endf,
        (startf * np.float32(FRAME_MS)).astype(np.float32),
        (endf * np.float32(FRAME_MS)).astype(np.float32),
        confv,
    )


def kernel(
    attentions,
    audio_start_idx=AUDIO_START,
    audio_end_idx=AUDIO_END,
    text_start_idx=TEXT_START,
    **_unused,
):
    attn = np.asarray(attentions, dtype=np.float32)
    a0 = int(np.asarray(audio_start_idx))
    a1 = int(np.asarray(audio_end_idx))
    t0 = int(np.asarray(text_start_idx))
    if attn.shape != (L, B, H, S, S) or (a0, a1, t0) != (
        AUDIO_START,
        AUDIO_END,
        TEXT_START,
    ):
        return _reference_numpy(attn, a0, a1, t0)
    in_maps = _prep_in_maps(attn)
    try:
        res = _run(in_maps)
    except Exception:
        try:
            res = _run(in_maps)
        except Exception as ex:  # noqa: BLE001
            sys.stderr.write(f"kernel: device path failed ({ex!r}); CPU fallback\n")
            return _reference_numpy(attn, a0, a1, t0)
    return _assemble(res.results)
